# revision 14
# baseline (speedup 1.0000x reference)
"""HSMNet cost-volume + disparity softmax-regression on 8 Trainium2 NeuronCores.

Reference computation (per batch b):
  cost[c,d,h,w] = |ref[c,h,w] - tgt[c,h,w-d]| for w>=d else 0
  cost_agg[d,h,w] = sum_c cost
  pred[h,w] = sum_d d * softmax_d(cost_agg)

Sharding: 8 cores = 4 batches x 2 h-halves (40 rows of 80 each); PIX=6400
pixels per core, flattened.

v2 design (segment-packed, no host replication):
  - SBUF partition = c + 32*s where s = pixel segment (4 segments of 1600).
    ref [128,1600]; tgt loaded twice with 24- and 23-col left pads (tgte/
    tgto) so every disparity's column offset is 4B-aligned and DVE
    tensor_tensor stays in 2x_1P mode. Total input DMA ~1.2MB/core (vs
    3.3MB for the 4x-replicated layout).
  - per d: ONE DVE subtract [128,1600] (~0.9us), ACT Abs in-place
    (~0.5us, 4x mode), then 4 chunk matmuls reduce channels into a
    RESIDENT PSUM cost volume [96, 4 banks] with out row = 4d+s.
    The reduce lhsT is a sliding 96-col window of a single [128,188]
    matrix G (G[c+32s, 92+s]=1), so the weights are generated on-chip by
    5 memsets and only the LDWEIGHTS offset changes per d.
  - w<d mask: 4 matmuls add -100 at invalid (row, x%160<d) positions
    (exp -> ~0; reference's exp(0) terms are ~1e-15 of den for randn
    inputs). They open the PSUM accumulation (start=True) and run early,
    also warming the PE HAM clock-gate during the input DMA window.
  - tail split in column halves: exp (PSUM->bf16 E), den/num matmul with
    lnd [96,8] (den col s = sum_d E[4d+s], num col 4+s weights by d),
    PSUM->SBUF copy (DVE for half A, ACT for half B), out DMA per half.
  - first two even d's are computed in column halves gated on split input
    DMAs so DVE starts ~1us earlier; odd d's run last so tgto can land
    late. The last two d's use DVE bitand for abs so the tail isn't
    gated on ACT.
  - host divides num/den.
"""
import os
import sys
import threading

for _p in ("/opt/trn_rl_repo",):
    if os.path.isdir(_p) and _p not in sys.path:
        sys.path.insert(0, _p)

import numpy as np
import ml_dtypes

import concourse.bacc as bacc
import concourse.mybir as mybir
from concourse.tile import TileContext
from concourse.bass_utils import run_bass_kernel_spmd

dt = mybir.dt

# problem shape (hardcoded per spec)
B, C, H, W = 4, 32, 80, 160
D = 24
HP = H // 2            # rows per core
PIX = HP * W           # 6400 pixels per core
SEG = 4                # pixel segments packed into partitions
SW = PIX // SEG        # 1600 segment width (free dim)
CH = 400               # matmul chunk (one 512-f32 PSUM bank each)
PADE = 24              # left pad of tgte (even-d tile)
PADO = 23              # left pad of tgto (odd-d tile)
GA = 92                # sliding-window anchor col in G (= 4*(D-1))
N_CORES = 8
MBIG = -100.0          # mask penalty added to invalid cost entries

DIFF_BUFS = int(os.environ.get("HSM_DIFF_BUFS", "6"))

# d-processing order: evens first (tgte), odds later (tgto lands late)
D_ORDER = list(range(0, D, 2)) + list(range(1, D, 2))

# Non-last d's are processed in PAIRS sharing one [128, 3200] diff tile so
# a single abs op covers both (halving the per-op overhead: ACT ACTIVATE
# costs (FD+352)/1.2 at 1x, DVE bitand (58+FD/4)/0.96 at 4x).  Engine per
# pair: 8 ACT-pairs / 3 DVE-pairs balances ACT (~23.7us) against the DVE
# sub chain (~23.6us + bitands).
_DEF_ABS = ["act", "act", "dve"] * 3 + ["act", "act"]
ABS_ENGINES = os.environ.get("HSM_ABS", ",".join(_DEF_ABS)).split(",")


def _build_program():
    nc = bacc.Bacc("TRN2", target_bir_lowering=False)
    ref_h = nc.dram_tensor("ref", [128, SW], dt.float16, kind="ExternalInput")
    tgte_h = nc.dram_tensor("tgte", [128, PADE + SW], dt.float16,
                            kind="ExternalInput")
    tgto_h = nc.dram_tensor("tgto", [128, PADO + SW], dt.float16,
                            kind="ExternalInput")
    mk_h = nc.dram_tensor("mk", [D, 96], dt.float16, kind="ExternalInput")
    mbin_h = nc.dram_tensor("mbin", [D, 2048], dt.float16, kind="ExternalInput")
    lnd_h = nc.dram_tensor("lnd", [96, 8], dt.bfloat16, kind="ExternalInput")
    out_h = nc.dram_tensor("out", [8, SW], dt.float32, kind="ExternalOutput")

    with TileContext(nc) as tc:
        with tc.tile_pool(name="const", bufs=1) as cpool, \
             tc.tile_pool(name="inp", bufs=1) as ipool, \
             tc.tile_pool(name="diffp", bufs=DIFF_BUFS) as dpool, \
             tc.tile_pool(name="ep", bufs=1) as epool, \
             tc.tile_pool(name="ps", bufs=1, space="PSUM") as ppool:

            G = cpool.tile([128, GA + 96], dt.float16, name="G")
            mk_sb = cpool.tile([D, 96], dt.float16, name="mk")
            mbin_sb = cpool.tile([D, 2048], dt.float16, name="mbin")
            lnd_sb = cpool.tile([96, 8], dt.bfloat16, name="lnd")

            ref = ipool.tile([128, SW], dt.float16, name="ref")
            tgte = ipool.tile([128, PADE + SW], dt.float16, name="tgte")
            tgto = ipool.tile([128, PADO + SW], dt.float16, name="tgto")

            # --- DMA issue first (DMA ring spin-up is ~1.5-2.5us after
            # dispatch, so dispatch before any on-chip memsets).  DMA cost
            # is descriptor(row)-dominated, so no column splits; ref rows
            # are split across rings so ref and tgte finish together.
            # sync ring = 16 SDMA engines, gpsimd = 8, scalar = 4.  tgto is
            # only needed when the odd-d phase starts ~13us in; consts
            # (mk+mbin) gate only the mask matmuls, lnd only the tail. ---
            nc.sync.dma_start(tgte[:], tgte_h[:])
            nc.gpsimd.dma_start(ref[16:128, :], ref_h[16:128, :])
            nc.sync.dma_start(ref[0:16, :], ref_h[0:16, :])
            nc.gpsimd.dma_start(tgto[:], tgto_h[:])
            nc.scalar.dma_start(mk_sb[:], mk_h[:])
            nc.scalar.dma_start(mbin_sb[:], mbin_h[:])
            nc.scalar.dma_start(lnd_sb[:], lnd_h[:])

            # --- on-chip reduce weights: G[c+32s, GA+s] = 1 ---
            nc.gpsimd.memset(G[:], 0.0)
            for s in range(SEG):
                nc.gpsimd.memset(G[32 * s:32 * s + 32, GA + s:GA + s + 1], 1.0)

            # --- PSUM: whole cost volume resident; row 4d+s ---
            costA = ppool.tile([96, 1024], dt.float32, name="costA")
            costB = ppool.tile([96, 1024], dt.float32, name="costB")
            ndA = ppool.tile([8, 1024], dt.float32, name="ndA")
            ndB = ppool.tile([8, 1024], dt.float32, name="ndB")

            # mask matmuls open each bank's accumulation group (and warm
            # the PE clock-gate early, during the input DMA window)
            for k in range(4):
                tgt_ps = (costA if k < 2 else costB)
                nc.tensor.matmul(
                    tgt_ps[:, 512 * (k % 2):512 * (k % 2) + 512],
                    mk_sb[:], mbin_sb[:, 512 * k:512 * k + 512],
                    start=True, stop=False)

            def emit_mms(d, ks, src, x0=0):
                lhsT = G[:, GA - 4 * d:GA - 4 * d + 96]
                last = D_ORDER[-1]
                for k in ks:
                    tgt_ps = (costA if k < 2 else costB)
                    nc.tensor.matmul(
                        tgt_ps[:, 512 * (k % 2):512 * (k % 2) + CH],
                        lhsT, src[:, x0 + CH * k:x0 + CH * k + CH],
                        start=False, stop=(d == last))

            def tgt_src(d):
                if d % 2 == 0:
                    return tgte[:, PADE - d:PADE - d + SW]
                return tgto[:, PADO - d:PADO - d + SW]

            # 11 pairs + one single (d=21) + the cascaded last d (23)
            pairs = [(D_ORDER[2 * i], D_ORDER[2 * i + 1]) for i in range(11)]
            for i, (da, db) in enumerate(pairs):
                pt = dpool.tile([128, 2 * SW], dt.float16, tag="diff",
                                name=f"diffp_{da}_{db}")
                for h, dd in enumerate((da, db)):
                    nc.vector.tensor_tensor(pt[:, SW * h:SW * (h + 1)],
                                            ref[:], tgt_src(dd),
                                            mybir.AluOpType.subtract)
                if ABS_ENGINES[i % len(ABS_ENGINES)] == "act":
                    nc.scalar.activation(pt[:], pt[:],
                                         mybir.ActivationFunctionType.Abs)
                else:
                    du = pt[:].bitcast(dt.uint16)
                    nc.vector.tensor_scalar(du, du, 0x7FFF, None,
                                            mybir.AluOpType.bitwise_and)
                for h, dd in enumerate((da, db)):
                    emit_mms(dd, range(4), pt, x0=SW * h)

            # single d=D_ORDER[22]
            ds = D_ORDER[22]
            st = dpool.tile([128, SW], dt.float16, tag="diff", name="diff_s")
            nc.vector.tensor_tensor(st[:], ref[:], tgt_src(ds),
                                    mybir.AluOpType.subtract)
            du = st[:].bitcast(dt.uint16)
            nc.vector.tensor_scalar(du, du, 0x7FFF, None,
                                    mybir.AluOpType.bitwise_and)
            emit_mms(ds, range(4), st)

            # last d: interleave half-width sub/bitand/matmuls so the exp
            # tail cascades earlier
            dl = D_ORDER[23]
            lt = dpool.tile([128, SW], dt.float16, tag="diff", name="diff_l")
            for h in range(2):
                x0, x1 = h * (SW // 2), (h + 1) * (SW // 2)
                ts = tgt_src(dl)
                nc.vector.tensor_tensor(lt[:, x0:x1], ref[:, x0:x1],
                                        ts[:, x0:x1],
                                        mybir.AluOpType.subtract)
                du = lt[:, x0:x1].bitcast(dt.uint16)
                nc.vector.tensor_scalar(du, du, 0x7FFF, None,
                                        mybir.AluOpType.bitwise_and)
                emit_mms(dl, (2 * h, 2 * h + 1), lt)

            # --- tail: per column half: exp -> den/num matmul -> copy ->
            # out DMA.  Half A copies on DVE, half B on ACT, so the two
            # halves' tails overlap. ---
            for h, (cost, ndt) in enumerate(((costA, ndA), (costB, ndB))):
                E = epool.tile([96, 800], dt.bfloat16, name=f"E{h}")
                src = cost[:].rearrange("p (k x) -> p k x", k=2)[:, :, 0:CH]
                dst = E[:].rearrange("p (k x) -> p k x", x=CH)
                nc.scalar.activation(dst, src,
                                     mybir.ActivationFunctionType.Exp)
                for k in range(2):
                    nc.tensor.matmul(ndt[:, 512 * k:512 * k + CH],
                                     lnd_sb[:], E[:, CH * k:CH * k + CH],
                                     start=True, stop=True)
                out_sb = epool.tile([8, 800], dt.float32, name=f"osb{h}")
                osv = out_sb[:].rearrange("p (k x) -> p k x", x=CH)
                ndsrc = ndt[:].rearrange("p (k x) -> p k x", k=2)[:, :, 0:CH]
                if h == 0:
                    nc.vector.tensor_copy(osv, ndsrc)
                else:
                    nc.scalar.activation(osv, ndsrc,
                                         mybir.ActivationFunctionType.Copy)
                nc.sync.dma_start(out_h[:, 800 * h:800 * (h + 1)], out_sb[:])

    nc.compile()
    return nc


def _host_constants():
    dr = np.arange(D)
    s4 = np.arange(SEG)
    # mk[dr, 4*dr+s] = MBIG
    mk = np.zeros((D, 96), np.float16)
    for s in range(SEG):
        mk[dr, 4 * dr + s] = MBIG
    # mbin[dr, 512k + j] = 1 if j<400 and ((400k+j) mod 160) < dr
    mbin = np.zeros((D, 2048), np.float16)
    for k in range(4):
        j = np.arange(CH)
        valid = ((400 * k + j) % W)[None, :] < dr[:, None]
        mbin[:, 512 * k:512 * k + CH] = valid.astype(np.float16)
    # lnd[4d+s, s] = 1 ; lnd[4d+s, 4+s] = d
    lnd = np.zeros((96, 8), np.float32)
    for d in range(D):
        for s in range(SEG):
            lnd[4 * d + s, s] = 1.0
            lnd[4 * d + s, 4 + s] = d
    lnd = lnd.astype(ml_dtypes.bfloat16)
    return mk, mbin, lnd


_lock = threading.Lock()
_cache = {}


def _get_program():
    with _lock:
        if "nc" not in _cache:
            _cache["nc"] = _build_program()
            _cache["consts"] = _host_constants()
        return _cache["nc"], _cache["consts"]


def _run(refimg_fea, targetimg_fea, trace=False):
    nc, (mk, mbin, lnd) = _get_program()
    ref = np.asarray(refimg_fea, dtype=np.float32).astype(np.float16)
    tgt = np.asarray(targetimg_fea, dtype=np.float32).astype(np.float16)
    in_maps = []
    for core in range(N_CORES):
        b, hh = core // 2, core % 2
        refc = ref[b, :, HP * hh:HP * (hh + 1), :].reshape(C, PIX)
        tgtc = tgt[b, :, HP * hh:HP * (hh + 1), :].reshape(C, PIX)
        # segment packing: row c+32s carries pixels [1600s, 1600s+1600)
        ref_seg = refc.reshape(C, SEG, SW).transpose(1, 0, 2).reshape(128, SW)
        tpe = np.zeros((C, PADE + PIX), np.float16)
        tpe[:, PADE:] = tgtc
        tpo = np.zeros((C, PADO + PIX), np.float16)
        tpo[:, PADO:] = tgtc
        tgte = np.stack([tpe[:, SW * s:SW * s + PADE + SW]
                         for s in range(SEG)]).reshape(128, PADE + SW)
        tgto = np.stack([tpo[:, SW * s:SW * s + PADO + SW]
                         for s in range(SEG)]).reshape(128, PADO + SW)
        in_maps.append({
            "ref": ref_seg, "tgte": tgte, "tgto": tgto,
            "mk": mk, "mbin": mbin, "lnd": lnd,
        })
    res = run_bass_kernel_spmd(nc, in_maps, core_ids=list(range(N_CORES)),
                               trace=trace)
    out = np.empty((B, H, W), np.float32)
    for core in range(N_CORES):
        b, hh = core // 2, core % 2
        nd = res.results[core]["out"]          # [8, 1600]: den rows 0-3 (seg), num rows 4-7
        pred = nd[4:8] / nd[0:4]               # [4, 1600] -> flat 6400
        out[b, HP * hh:HP * (hh + 1), :] = pred.reshape(HP, W)
    return out, res


def kernel(refimg_fea, targetimg_fea, maxdisp):
    assert int(maxdisp) == D, f"kernel hardcodes maxdisp={D}, got {maxdisp}"
    out, _ = _run(refimg_fea, targetimg_fea)
    return out


# revision 17
# speedup vs baseline: 1.0652x; 1.0652x over previous
"""HSMNet cost-volume + disparity softmax-regression on 8 Trainium2 NeuronCores.

Reference computation (per batch b):
  cost[c,d,h,w] = |ref[c,h,w] - tgt[c,h,w-d]| for w>=d else 0
  cost_agg[d,h,w] = sum_c cost
  pred[h,w] = sum_d d * softmax_d(cost_agg)

Sharding: 8 cores = 4 batches x 2 h-halves (40 rows of 80 each); PIX=6400
pixels per core, flattened.

v2 design (segment-packed, no host replication):
  - SBUF partition = c + 32*s where s = pixel segment (4 segments of 1600).
    ref [128,1600]; tgt loaded twice with 24- and 23-col left pads (tgte/
    tgto) so every disparity's column offset is 4B-aligned and DVE
    tensor_tensor stays in 2x_1P mode. Total input DMA ~1.2MB/core (vs
    3.3MB for the 4x-replicated layout).
  - per d: ONE DVE subtract [128,1600] (~0.9us), ACT Abs in-place
    (~0.5us, 4x mode), then 4 chunk matmuls reduce channels into a
    RESIDENT PSUM cost volume [96, 4 banks] with out row = 4d+s.
    The reduce lhsT is a sliding 96-col window of a single [128,188]
    matrix G (G[c+32s, 92+s]=1), so the weights are generated on-chip by
    5 memsets and only the LDWEIGHTS offset changes per d.
  - w<d mask: 4 matmuls add -100 at invalid (row, x%160<d) positions
    (exp -> ~0; reference's exp(0) terms are ~1e-15 of den for randn
    inputs). They open the PSUM accumulation (start=True) and run early,
    also warming the PE HAM clock-gate during the input DMA window.
  - tail split in column halves: exp (PSUM->bf16 E), den/num matmul with
    lnd [96,8] (den col s = sum_d E[4d+s], num col 4+s weights by d),
    PSUM->SBUF copy (DVE for half A, ACT for half B), out DMA per half.
  - first two even d's are computed in column halves gated on split input
    DMAs so DVE starts ~1us earlier; odd d's run last so tgto can land
    late. The last two d's use DVE bitand for abs so the tail isn't
    gated on ACT.
  - host divides num/den.
"""
import os
import sys
import threading

for _p in ("/opt/trn_rl_repo",):
    if os.path.isdir(_p) and _p not in sys.path:
        sys.path.insert(0, _p)

import numpy as np
import ml_dtypes

import concourse.bacc as bacc
import concourse.mybir as mybir
from concourse.tile import TileContext
from concourse.bass_utils import run_bass_kernel_spmd

dt = mybir.dt

# problem shape (hardcoded per spec)
B, C, H, W = 4, 32, 80, 160
D = 24
HP = H // 2            # rows per core
PIX = HP * W           # 6400 pixels per core
SEG = 4                # pixel segments packed into partitions
SW = PIX // SEG        # 1600 segment width (free dim)
CH = 400               # matmul chunk (one 512-f32 PSUM bank each)
PADE = 24              # left pad of tgte (even-d tile)
PADO = 23              # left pad of tgto (odd-d tile)
GA = 92                # sliding-window anchor col in G (= 4*(D-1))
N_CORES = 8
MBIG = -100.0          # mask penalty added to invalid cost entries

DIFF_BUFS = int(os.environ.get("HSM_DIFF_BUFS", "8"))

# d-processing order: evens first (tgte), odds later (tgto lands late)
D_ORDER = list(range(0, D, 2)) + list(range(1, D, 2))

# abs engine per d-loop index: ACT ACTIVATE Abs runs at 1x (~1.63us for
# FD=1600) vs DVE sub ~0.90us + DVE bitand ~0.57us (4x); ~15 on ACT and
# the rest on DVE balances the two chains.  The last d is cascaded on DVE.
_DEF_ABS = ["act", "act", "dve"] * 7 + ["act", "dve", "dve"]
ABS_ENGINES = os.environ.get("HSM_ABS", ",".join(_DEF_ABS)).split(",")


def _build_program():
    nc = bacc.Bacc("TRN2", target_bir_lowering=False)
    ref_h = nc.dram_tensor("ref", [128, SW], dt.float16, kind="ExternalInput")
    tgte_h = nc.dram_tensor("tgte", [128, PADE + SW], dt.float16,
                            kind="ExternalInput")
    tgto_h = nc.dram_tensor("tgto", [128, PADO + SW], dt.float16,
                            kind="ExternalInput")
    mk_h = nc.dram_tensor("mk", [D, 96], dt.float16, kind="ExternalInput")
    mbin_h = nc.dram_tensor("mbin", [D, 2048], dt.float16, kind="ExternalInput")
    lnd_h = nc.dram_tensor("lnd", [96, 8], dt.bfloat16, kind="ExternalInput")
    out_h = nc.dram_tensor("out", [8, SW], dt.float32, kind="ExternalOutput")

    with TileContext(nc) as tc:
        with tc.tile_pool(name="const", bufs=1) as cpool, \
             tc.tile_pool(name="inp", bufs=1) as ipool, \
             tc.tile_pool(name="diffp", bufs=DIFF_BUFS) as dpool, \
             tc.tile_pool(name="ep", bufs=1) as epool, \
             tc.tile_pool(name="ps", bufs=1, space="PSUM") as ppool:

            G = cpool.tile([128, GA + 96], dt.float16, name="G")
            mk_sb = cpool.tile([D, 96], dt.float16, name="mk")
            mbin_sb = cpool.tile([D, 2048], dt.float16, name="mbin")
            lnd_sb = cpool.tile([96, 8], dt.bfloat16, name="lnd")

            ref = ipool.tile([128, SW], dt.float16, name="ref")
            tgte = ipool.tile([128, PADE + SW], dt.float16, name="tgte")
            tgto = ipool.tile([128, PADO + SW], dt.float16, name="tgto")

            # --- DMA issue first (DMA ring spin-up is ~1.5-2.5us after
            # dispatch, so dispatch before any on-chip memsets).  The first
            # two even d's are computed in column halves, so tgte/ref are
            # loaded as column halves (A-halves first on both rings) and
            # DVE starts ~2us before the full tiles land.  sync ring = 16
            # SDMA engines, gpsimd = 8, scalar = 4.  tgto is only needed
            # when the odd-d phase starts ~13us in; consts (mk+mbin) gate
            # only the mask matmuls, lnd only the tail. ---
            E1 = PADE + SW // 2          # 824: covers A-half reads of all even d
            nc.sync.dma_start(tgte[:, 0:E1], tgte_h[:, 0:E1])
            nc.gpsimd.dma_start(ref[:, 0:SW // 2], ref_h[:, 0:SW // 2])
            nc.sync.dma_start(tgte[:, E1:], tgte_h[:, E1:])
            nc.gpsimd.dma_start(ref[:, SW // 2:], ref_h[:, SW // 2:])
            nc.sync.dma_start(tgto[:], tgto_h[:])
            nc.scalar.dma_start(mk_sb[:], mk_h[:])
            nc.scalar.dma_start(mbin_sb[:], mbin_h[:])
            nc.scalar.dma_start(lnd_sb[:], lnd_h[:])

            # --- on-chip reduce weights: G[c+32s, GA+s] = 1 ---
            nc.gpsimd.memset(G[:], 0.0)
            for s in range(SEG):
                nc.gpsimd.memset(G[32 * s:32 * s + 32, GA + s:GA + s + 1], 1.0)

            # --- PSUM: whole cost volume resident; row 4d+s ---
            costA = ppool.tile([96, 1024], dt.float32, name="costA")
            costB = ppool.tile([96, 1024], dt.float32, name="costB")
            ndA = ppool.tile([8, 1024], dt.float32, name="ndA")
            ndB = ppool.tile([8, 1024], dt.float32, name="ndB")

            # mask matmuls open each bank's accumulation group (and warm
            # the PE clock-gate early, during the input DMA window)
            for k in range(4):
                tgt_ps = (costA if k < 2 else costB)
                nc.tensor.matmul(
                    tgt_ps[:, 512 * (k % 2):512 * (k % 2) + 512],
                    mk_sb[:], mbin_sb[:, 512 * k:512 * k + 512],
                    start=True, stop=False)

            def emit_mms(d, ks, src, x0=0):
                lhsT = G[:, GA - 4 * d:GA - 4 * d + 96]
                last = D_ORDER[-1]
                for k in ks:
                    tgt_ps = (costA if k < 2 else costB)
                    nc.tensor.matmul(
                        tgt_ps[:, 512 * (k % 2):512 * (k % 2) + CH],
                        lhsT, src[:, x0 + CH * k:x0 + CH * k + CH],
                        start=False, stop=(d == last))

            def tgt_src(d):
                if d % 2 == 0:
                    return tgte[:, PADE - d:PADE - d + SW]
                return tgto[:, PADO - d:PADO - d + SW]

            diffs = {}
            # first two even d's in column halves, interleaved, so DVE
            # starts on the A-half input DMAs ~2us before full tiles land
            sd = D_ORDER[:2]
            for half in range(2):
                x0, x1 = half * (SW // 2), (half + 1) * (SW // 2)
                for d in sd:
                    if half == 0:
                        diffs[d] = dpool.tile([128, SW], dt.float16,
                                              tag="diff", name=f"diff_{d}")
                    ts = tgt_src(d)
                    nc.vector.tensor_tensor(diffs[d][:, x0:x1],
                                            ref[:, x0:x1], ts[:, x0:x1],
                                            mybir.AluOpType.subtract)
                    nc.scalar.activation(diffs[d][:, x0:x1],
                                         diffs[d][:, x0:x1],
                                         mybir.ActivationFunctionType.Abs)
                    emit_mms(d, (2 * half, 2 * half + 1), diffs[d])

            for i, d in enumerate(D_ORDER[2:], start=2):
                diffs[d] = dpool.tile([128, SW], dt.float16, tag="diff",
                                      name=f"diff_{d}")
                if i == D - 1:
                    # last d: interleave half-width sub/bitand/matmuls so
                    # the exp tail cascades earlier
                    for h in range(2):
                        x0, x1 = h * (SW // 2), (h + 1) * (SW // 2)
                        ts = tgt_src(d)
                        nc.vector.tensor_tensor(diffs[d][:, x0:x1],
                                                ref[:, x0:x1], ts[:, x0:x1],
                                                mybir.AluOpType.subtract)
                        du = diffs[d][:, x0:x1].bitcast(dt.uint16)
                        nc.vector.tensor_scalar(du, du, 0x7FFF, None,
                                                mybir.AluOpType.bitwise_and)
                        emit_mms(d, (2 * h, 2 * h + 1), diffs[d])
                    continue
                nc.vector.tensor_tensor(diffs[d][:], ref[:], tgt_src(d),
                                        mybir.AluOpType.subtract)
                if ABS_ENGINES[i % len(ABS_ENGINES)] == "act":
                    nc.scalar.activation(diffs[d][:], diffs[d][:],
                                         mybir.ActivationFunctionType.Abs)
                else:
                    du = diffs[d][:].bitcast(dt.uint16)
                    nc.vector.tensor_scalar(du, du, 0x7FFF, None,
                                            mybir.AluOpType.bitwise_and)
                emit_mms(d, range(4), diffs[d])

            # --- tail: per column half: exp -> den/num matmul -> copy ->
            # out DMA.  Half A copies on DVE, half B on ACT, so the two
            # halves' tails overlap. ---
            for h, (cost, ndt) in enumerate(((costA, ndA), (costB, ndB))):
                E = epool.tile([96, 800], dt.bfloat16, name=f"E{h}")
                src = cost[:].rearrange("p (k x) -> p k x", k=2)[:, :, 0:CH]
                dst = E[:].rearrange("p (k x) -> p k x", x=CH)
                nc.scalar.activation(dst, src,
                                     mybir.ActivationFunctionType.Exp)
                for k in range(2):
                    nc.tensor.matmul(ndt[:, 512 * k:512 * k + CH],
                                     lnd_sb[:], E[:, CH * k:CH * k + CH],
                                     start=True, stop=True)
                out_sb = epool.tile([8, 800], dt.float32, name=f"osb{h}")
                osv = out_sb[:].rearrange("p (k x) -> p k x", x=CH)
                ndsrc = ndt[:].rearrange("p (k x) -> p k x", k=2)[:, :, 0:CH]
                if h == 0:
                    nc.vector.tensor_copy(osv, ndsrc)
                else:
                    nc.scalar.activation(osv, ndsrc,
                                         mybir.ActivationFunctionType.Copy)
                nc.sync.dma_start(out_h[:, 800 * h:800 * (h + 1)], out_sb[:])

    nc.compile()
    return nc


def _host_constants():
    dr = np.arange(D)
    s4 = np.arange(SEG)
    # mk[dr, 4*dr+s] = MBIG
    mk = np.zeros((D, 96), np.float16)
    for s in range(SEG):
        mk[dr, 4 * dr + s] = MBIG
    # mbin[dr, 512k + j] = 1 if j<400 and ((400k+j) mod 160) < dr
    mbin = np.zeros((D, 2048), np.float16)
    for k in range(4):
        j = np.arange(CH)
        valid = ((400 * k + j) % W)[None, :] < dr[:, None]
        mbin[:, 512 * k:512 * k + CH] = valid.astype(np.float16)
    # lnd[4d+s, s] = 1 ; lnd[4d+s, 4+s] = d
    lnd = np.zeros((96, 8), np.float32)
    for d in range(D):
        for s in range(SEG):
            lnd[4 * d + s, s] = 1.0
            lnd[4 * d + s, 4 + s] = d
    lnd = lnd.astype(ml_dtypes.bfloat16)
    return mk, mbin, lnd


_lock = threading.Lock()
_cache = {}


def _get_program():
    with _lock:
        if "nc" not in _cache:
            _cache["nc"] = _build_program()
            _cache["consts"] = _host_constants()
        return _cache["nc"], _cache["consts"]


def _run(refimg_fea, targetimg_fea, trace=False):
    nc, (mk, mbin, lnd) = _get_program()
    ref = np.asarray(refimg_fea, dtype=np.float32).astype(np.float16)
    tgt = np.asarray(targetimg_fea, dtype=np.float32).astype(np.float16)
    in_maps = []
    for core in range(N_CORES):
        b, hh = core // 2, core % 2
        refc = ref[b, :, HP * hh:HP * (hh + 1), :].reshape(C, PIX)
        tgtc = tgt[b, :, HP * hh:HP * (hh + 1), :].reshape(C, PIX)
        # segment packing: row c+32s carries pixels [1600s, 1600s+1600)
        ref_seg = refc.reshape(C, SEG, SW).transpose(1, 0, 2).reshape(128, SW)
        tpe = np.zeros((C, PADE + PIX), np.float16)
        tpe[:, PADE:] = tgtc
        tpo = np.zeros((C, PADO + PIX), np.float16)
        tpo[:, PADO:] = tgtc
        tgte = np.stack([tpe[:, SW * s:SW * s + PADE + SW]
                         for s in range(SEG)]).reshape(128, PADE + SW)
        tgto = np.stack([tpo[:, SW * s:SW * s + PADO + SW]
                         for s in range(SEG)]).reshape(128, PADO + SW)
        in_maps.append({
            "ref": ref_seg, "tgte": tgte, "tgto": tgto,
            "mk": mk, "mbin": mbin, "lnd": lnd,
        })
    res = run_bass_kernel_spmd(nc, in_maps, core_ids=list(range(N_CORES)),
                               trace=trace)
    out = np.empty((B, H, W), np.float32)
    for core in range(N_CORES):
        b, hh = core // 2, core % 2
        nd = res.results[core]["out"]          # [8, 1600]: den rows 0-3 (seg), num rows 4-7
        pred = nd[4:8] / nd[0:4]               # [4, 1600] -> flat 6400
        out[b, HP * hh:HP * (hh + 1), :] = pred.reshape(HP, W)
    return out, res


def kernel(refimg_fea, targetimg_fea, maxdisp):
    assert int(maxdisp) == D, f"kernel hardcodes maxdisp={D}, got {maxdisp}"
    out, _ = _run(refimg_fea, targetimg_fea)
    return out
